# revision 39
# baseline (speedup 1.0000x reference)
"""Trainium2 Bass kernel for nn_MultiHeadAttention_83056077570808.

GQA multi-head attention (32 q heads, 8 kv heads, d_head=128, T=2048,
D=4096) with RoPE, tanh soft-capping at 30, causal mask, fp32 reference.

Sharding: tensor-parallel over heads across 8 cores. Core c owns kv head c
and q heads 4c..4c+3: Wq/Wk/Wv column-sharded, Wo row-sharded; activations
replicated. Each core computes a partial output (its heads' contribution
through its Wo rows); the host sums the 8 partials.

Schedule (v3; cost-model-guided, 515us baseline -> 447us):
  - the DMA engine is effectively a single serial resource (~330 B/ns for
    contiguous-run transfers), so every multi-MB load is split into ~1MB
    pieces emitted just-in-time in the same order the consuming PE units
    pop; weights are pre-rearranged on the HOST into the SBUF tile layout
    (head-major wq, k-major wk/wv) so their DMA runs are 8KB-contiguous
  - minimal prefix: K chunks 0..1 (+RoPE), V chunks 0..1, Q proj of chunk
    0; K2..7 / V2..7 stream as pacer units inside the attention loop at
    their causal deadlines
  - causal diagonal handled exactly: QK/tanh/exp sliced to the unmasked
    query range per 128-tile, a [128,128] triangular bf16 DVE multiply
    masks only the diagonal block (PV blocks fully under the diagonal are
    skipped), replacing the baseline's Pool-engine mask of whole groups
  - a deadline-tagged pacer interleaves filler units (Q proj half-chains,
    O proj (s4,nch) chains, K/V chunks, xq quarter-slab prefetches)
    BETWEEN each group's exp and PV so PE keeps running while ACT owns
    the tanh/exp chain; only units due this chunk count toward the tick
    budget, letting O proj drift into the ACT-bound late chunks
  - RoPE rotation matmul uses a bf16 rotation matrix for q (qraw held in
    bf16) and f32r for k; cos/sin tables are bf16

Per-core pipeline layouts (contraction dim = partition dim everywhere):
  - host supplies query/key/value pre-transposed as X^T [D, T] in bf16
  - K/V proj: kT/vT [dk, T] = Wk/Wv-tile.T @ X^T   (bf16 matmuls, fp32 acc)
  - RoPE on kT and qT via a 128x128 rotation matmul + cos/sin elementwise
  - V transposed on-PE to V [T, dk] bf16, augmented with a ones column so
    the PV matmul computes the softmax denominator for free
  - S^T[Tt, t] = kT_rope-tile.T @ qT_rope      (f32r)
    tanh in PSUM, exp to bf16 SBUF (ACT scales fused)
  - attn[t, 0:129] = P^T-slice.T @ [V | ones]  (bf16, accumulated over
    T tiles; col 128 = denominator); normalize, PE-transpose to attnT
  - O proj: out[t, :] += attnT-tile.T @ Wo-tile (bf16)

No max-subtraction in softmax: capping bounds logits to [-30, 30].
Masked logits never reach attn: fully-masked PV blocks are skipped and
the diagonal 128x128 block is zeroed in bf16 after exp.
"""

import collections
import os
import sys

for _p in ("/opt/trn_rl_repo", os.path.expanduser("~/.axon_site/_ro/trn_rl_repo")):
    if os.path.isdir(_p) and _p not in sys.path:
        sys.path.insert(0, _p)

import numpy as np
import ml_dtypes

import concourse.bass as bass
import concourse.tile as tile
from concourse import bacc, mybir
from concourse.bass_utils import run_bass_kernel_spmd

F32 = mybir.dt.float32
F32R = mybir.dt.float32r
BF16 = mybir.dt.bfloat16

D_MODEL = 4096
KEY_SIZE = 128
NUM_Q_HEADS = 32
NUM_KV_HEADS = 8
N_CORES = 8
NH = NUM_Q_HEADS // NUM_KV_HEADS  # q heads per core = 4
ATTN_MULT = 0.08838834764831845
CAP = 30.0

Tanh = mybir.ActivationFunctionType.Tanh
Exp = mybir.ActivationFunctionType.Exp

PROJ_DT = os.environ.get("MHA_PROJ_DT", "bf16")


class Pacer:
    """Deadline-tagged filler queue: units are (cost_ns, fn, deadline_tcx).

    tick(budget) pops FIFO until ~budget ns of unit cost has been emitted.
    drain_due(tcx) force-emits everything whose deadline has arrived.
    """

    def __init__(self):
        self.q = collections.deque()

    def add(self, cost, fn, deadline):
        self.q.append((cost, fn, deadline))

    def total(self):
        return sum(c for c, _, _ in self.q)

    def due_total(self, tcx):
        return sum(c for c, _, dl in self.q if dl <= tcx)

    def tick(self, budget):
        spent = 0.0
        while self.q and spent < budget:
            c, fn, _ = self.q.popleft()
            fn()
            spent += c
        return spent

    def drain_due(self, tcx):
        keep = collections.deque()
        while self.q:
            c, fn, dl = self.q.popleft()
            if dl <= tcx:
                fn()
            else:
                keep.append((c, fn, dl))
        self.q = keep

    def drain_all(self):
        while self.q:
            _, fn, _ = self.q.popleft()
            fn()


def build_nc(T: int, causal: bool, proj_dt: str = PROJ_DT):
    """Emit the Bass program for one core (SPMD: all cores run this)."""
    D = D_MODEL
    TC = 512                 # t-chunk width for attention
    NTC = T // TC            # t-chunks
    NTT = T // 128           # T tiles (key side)
    NDT = D // 128           # contraction tiles over d_model = 32
    JW = NH * KEY_SIZE       # per-core q/o width = 512
    GW = 2                   # T tiles per QK group (2 PSUM banks)
    PDT = BF16 if proj_dt == "bf16" else F32R

    # pacer unit cost estimates (ns, full-pstate PE)
    C_QPROJ_HALF = 3400.0
    C_OPROJ_UNIT = 900.0
    C_VCHUNK = 3000.0
    C_KCHUNK = 3600.0

    nc = bacc.Bacc(None, target_bir_lowering=False)

    xq = nc.dram_tensor("xq", [D, T], PDT, kind="ExternalInput")
    xk = nc.dram_tensor("xk", [D, T], PDT, kind="ExternalInput")
    xv = nc.dram_tensor("xv", [D, T], PDT, kind="ExternalInput")
    wq = nc.dram_tensor("wq", [NH, 128, D // 128, 128], PDT, kind="ExternalInput")
    wk = nc.dram_tensor("wk", [128, D // 128, KEY_SIZE], PDT, kind="ExternalInput")
    wv = nc.dram_tensor("wv", [128, D // 128, KEY_SIZE], PDT, kind="ExternalInput")
    wo = nc.dram_tensor("wo", [JW, D], PDT, kind="ExternalInput")
    cosd = nc.dram_tensor("cosT", [128, T], BF16, kind="ExternalInput")
    sind = nc.dram_tensor("sinT", [128, T], BF16, kind="ExternalInput")
    rotd = nc.dram_tensor("rot", [128, 128], F32R, kind="ExternalInput")
    rotbd = nc.dram_tensor("rotb", [128, 128], BF16, kind="ExternalInput")
    identbd = nc.dram_tensor("identb", [128, 128], BF16, kind="ExternalInput")
    trid = nc.dram_tensor("tri", [128, 128], BF16, kind="ExternalInput")
    vbgd = nc.dram_tensor("vbg", [128, NTT, 4], BF16, kind="ExternalInput")
    outd = nc.dram_tensor("out", [T, D], BF16, kind="ExternalOutput")

    with tile.TileContext(nc) as tc:
        with (
            tc.tile_pool(name="const", bufs=1) as constp,
            tc.tile_pool(name="persist", bufs=1) as persist,
            tc.tile_pool(name="qkps", bufs=2, space="PSUM") as qkps,
            tc.tile_pool(name="pvps", bufs=1, space="PSUM") as pvps,
            tc.tile_pool(name="mmps", bufs=2, space="PSUM") as mmps,
        ):
            rot_sb = constp.tile([128, 128], F32R)
            rotb_sb = constp.tile([128, 128], BF16)
            identb_sb = constp.tile([128, 128], BF16)
            tri_sb = constp.tile([128, 128], BF16)
            cos_sb = constp.tile([128, T], BF16)
            sin_sb = constp.tile([128, T], BF16)

            kT_rope = persist.tile([128, T], F32R)
            vaug = persist.tile([128, NTT, 132], BF16)

            wpool = tc.alloc_tile_pool(name="wpool", bufs=1)
            wq_sb = wpool.tile([128, NH, NDT, 128], PDT)
            wo_sb = wpool.tile([128, NH, D], PDT)
            wk_sb = wpool.tile([128, NDT, 128], PDT)
            wv_sb = wpool.tile([128, NDT, 128], PDT)

            kvslab = tc.alloc_tile_pool(
                name="kvslab", bufs=int(os.environ.get("MHA_KVSLABS", "5"))
            )
            qslab = tc.alloc_tile_pool(
                name="qslab", bufs=int(os.environ.get("MHA_QSLABS", "4"))
            )
            qpool = tc.alloc_tile_pool(name="qpool", bufs=2)
            ropepool = tc.alloc_tile_pool(
                name="ropetmp", bufs=int(os.environ.get("MHA_RTBUFS", "2"))
            )
            vrawp = tc.alloc_tile_pool(name="vraw", bufs=2)

            # ---------------- helpers
            def load_qslabs(t0):
                """4 quarter-slabs [128, 8, TC] covering all D rows of xq
                for t-chunk at t0."""
                slabs = []
                for dh in range(4):
                    slab = qslab.tile([128, 8, TC], PDT, tag="qslab", name="qslab")
                    nc.sync.dma_start(
                        out=slab,
                        in_=xq[
                            dh * 1024 : (dh + 1) * 1024, t0 : t0 + TC
                        ].rearrange("(n k) t -> k n t", k=128),
                    )
                    slabs.append(slab)
                return slabs

            def rope(dst, src, t0, tw, rtile, f32r_src):
                """dst[128, tw] (f32r) = RoPE(src[128, tw]) at positions t0..

                In-place safe (dst may alias src): src is fully read by the
                rotation matmul and the cos-mul before dst is written."""
                rp = mmps.tile([128, 512], F32, tag="mm", name="rope_ps")
                nc.tensor.matmul(rp[:, :tw], rtile, src, start=True, stop=True)
                t1 = ropepool.tile([128, 512], F32, tag="rt", name="rope_t1")
                nc.gpsimd.tensor_mul(
                    t1[:, :tw],
                    src.bitcast(F32) if f32r_src else src,
                    cos_sb[:, t0 : t0 + tw],
                )
                t2 = ropepool.tile([128, 512], F32, tag="rt", name="rope_t2")
                nc.vector.tensor_mul(t2[:, :tw], rp[:, :tw], sin_sb[:, t0 : t0 + tw])
                nc.vector.tensor_add(dst, t1[:, :tw], t2[:, :tw])

            def kv_chunk(w_sb, xsrc, tch, dest_slice):
                """One 256-col projection chunk of K or V into dest slice."""
                ps = mmps.tile([128, 512], F32, tag="mm", name="kv_ps")
                for dh in range(4):
                    slab = kvslab.tile([128, 8, 256], PDT, tag="slab", name="kvslab")
                    nc.sync.dma_start(
                        out=slab,
                        in_=xsrc[
                            dh * 1024 : (dh + 1) * 1024,
                            tch * 256 : (tch + 1) * 256,
                        ].rearrange("(n k) t -> k n t", k=128),
                    )
                    for i in range(8):
                        nc.tensor.matmul(
                            ps[:, :256],
                            w_sb[:, dh * 8 + i, :],
                            slab[:, i, :],
                            start=(dh == 0 and i == 0),
                            stop=(dh == 3 and i == 7),
                        )
                nc.scalar.copy(out=dest_slice, in_=ps[:, :256])

            def k_chunk(tch):
                """K proj chunk into kT_rope (raw); rope runs in place after
                each odd chunk (pair 0's rope is emitted explicitly later so
                the first QK projections can slot in before cos/sin land)."""
                with nc.named_scope("kproj"):
                    kv_chunk(wk_sb, xk, tch, kT_rope[:, tch * 256 : (tch + 1) * 256])
                    if tch % 2 == 1 and tch >= 3:
                        ch = tch // 2
                        sl = kT_rope[:, ch * TC : (ch + 1) * TC]
                        rope(sl, sl, ch * TC, TC, rot_sb, True)

            def v_chunk(tch):
                """V proj chunk tch (256 cols) -> transpose -> vaug tiles."""
                with nc.named_scope("vproj"):
                    vraw = vrawp.tile([128, 256], BF16, tag="vr", name="vraw")
                    kv_chunk(wv_sb, xv, tch, vraw)
                    for b in (2 * tch, 2 * tch + 1):
                        tp = mmps.tile([128, 512], BF16, tag="mm", name="vtr_ps")
                        nc.tensor.transpose(
                            tp[:, :128],
                            vraw[:, (b % 2) * 128 : (b % 2 + 1) * 128],
                            identb_sb,
                        )
                        nc.vector.tensor_copy(vaug[:, b, 0:128], tp[:, :128])

            def qproj_half(slabs, ps_holder, qraw, jh, half):
                """Half a Q-proj chain: 16 matmuls (2 quarter-slabs)."""
                with nc.named_scope("qproj"):
                    if half == 0:
                        ps_holder[0] = mmps.tile([128, 512], F32, tag="mm", name="q_ps")
                    ps = ps_holder[0]
                    for dq in (2 * half, 2 * half + 1):
                        for i in range(8):
                            nc.tensor.matmul(
                                ps,
                                wq_sb[:, jh, dq * 8 + i, :],
                                slabs[dq][:, i, :],
                                start=(dq == 0 and i == 0),
                                stop=(dq == 3 and i == 7),
                            )
                    if half == 1:
                        nc.vector.tensor_copy(qraw[:, jh, :], ps)

            def qproj_full(slabs, qraw, jh, psum_pool=None):
                with nc.named_scope("qproj"):
                    if psum_pool is None:
                        ps = mmps.tile([128, 512], F32, tag="mm", name="q_ps")
                    else:
                        ps = psum_pool.tile([128, 2, 512], F32, tag="qk", name="q_ps")[
                            :, 0, :
                        ]
                    for dq in range(4):
                        for i in range(8):
                            nc.tensor.matmul(
                                ps,
                                wq_sb[:, jh, dq * 8 + i, :],
                                slabs[dq][:, i, :],
                                start=(dq == 0 and i == 0),
                                stop=(dq == 3 and i == 7),
                            )
                    nc.vector.tensor_copy(qraw[:, jh, :], ps)

            # ---------------- phase A prefix: K chunks 0..3 (+RoPE-k 0,1),
            # V chunks 0,1, Q proj(tc0). DMA emission order front-loads what
            # unlocks the most PE work per byte.
            nc.sync.dma_start(out=wk_sb, in_=wk[:])
            qraw0 = qpool.tile([128, NH, TC], BF16, tag="qraw", name="qraw")

            n_kch = T // 256
            n_kpre = 2 if causal else n_kch
            n_vpre = 2 if causal else n_kch

            def load_wq_head(jh):
                nc.sync.dma_start(out=wq_sb[:, jh], in_=wq[jh])

            k_chunk(0)
            k_chunk(1)
            load_wq_head(0)
            slabs0 = load_qslabs(0)
            load_wq_head(1)
            nc.sync.dma_start(out=rot_sb, in_=rotd[:])
            nc.sync.dma_start(out=rotb_sb, in_=rotbd[:])
            nc.sync.dma_start(out=cos_sb, in_=cosd[:])
            nc.sync.dma_start(out=sin_sb, in_=sind[:])
            nc.sync.dma_start(out=identb_sb, in_=identbd[:])
            nc.sync.dma_start(out=tri_sb, in_=trid[:])
            nc.sync.dma_start(out=vaug[:, :, 128:132], in_=vbgd[:])
            for tch in range(2, n_kpre):
                k_chunk(tch)
            qproj_full(slabs0, qraw0, 0, psum_pool=qkps)
            load_wq_head(2)
            qproj_full(slabs0, qraw0, 1, psum_pool=qkps)
            load_wq_head(3)
            rope(kT_rope[:, 0:TC], kT_rope[:, 0:TC], 0, TC, rot_sb, True)
            nc.sync.dma_start(out=wv_sb, in_=wv[:])
            qproj_full(slabs0, qraw0, 2, psum_pool=qkps)
            v_chunk(0)
            qproj_full(slabs0, qraw0, 3, psum_pool=qkps)
            v_chunk(1)
            for tch in range(2, n_vpre):
                v_chunk(tch)

            # ---------------- main loop
            nc.sync.dma_start(out=wo_sb, in_=wo.rearrange("(n k) d -> k n d", k=128))
            pacer = Pacer()
            pa = tc.alloc_tile_pool(name="pa", bufs=1)

            def make_oproj_units(attnT, t0):
                """O proj of a chunk: 32 (s4,nch) units, 4 head-matmuls each;
                osb half-rows [128, 2048] DMAed per 4 nch."""
                units = []
                osb_rows = [None] * 4

                def unit(s4, nch):
                    def f():
                        with nc.named_scope("oproj"):
                            if nch % 4 == 0:
                                osb_rows[s4] = pa.tile(
                                    [128, D // 2], BF16, tag="osb", bufs=2, name="osb"
                                )
                            ps = mmps.tile([128, 512], F32, tag="mm", name="o_ps")
                            for jh in range(NH):
                                nc.tensor.matmul(
                                    ps,
                                    attnT[:, jh, s4 * 128 : (s4 + 1) * 128],
                                    wo_sb[:, jh, nch * 512 : (nch + 1) * 512],
                                    start=(jh == 0),
                                    stop=(jh == NH - 1),
                                )
                            nc.vector.tensor_copy(
                                osb_rows[s4][:, (nch % 4) * 512 : (nch % 4 + 1) * 512],
                                ps,
                            )
                            if nch % 4 == 3:
                                half = nch // 4
                                nc.sync.dma_start(
                                    out=outd[
                                        t0 + s4 * 128 : t0 + (s4 + 1) * 128,
                                        half * (D // 2) : (half + 1) * (D // 2),
                                    ],
                                    in_=osb_rows[s4],
                                )

                    return f

                for s4 in range(4):
                    for nch in range(D // 512):
                        units.append(unit(s4, nch))
                return units

            def attn_fin(pv, attnT, h, s4):
                rc = pa.tile([128, 1], F32, tag="rc", bufs=4, name="rc")
                nc.vector.reciprocal(rc, pv[:, s4, 128:129])
                an = pa.tile([128, 128], BF16, tag="an", bufs=2, name="an")
                nc.vector.tensor_scalar_mul(an, pv[:, s4, 0:128], rc)
                tp = mmps.tile([128, 512], BF16, tag="mm", name="atr")
                nc.tensor.transpose(tp[:, :128], an, identb_sb)
                nc.vector.tensor_copy(
                    attnT[:, h, s4 * 128 : (s4 + 1) * 128], tp[:, :128]
                )

            qraw_cur = qraw0
            prev_attnT = None
            prev_t0 = 0
            slab_lists = {0: slabs0}

            def qslab_quarter_unit(idx, dh):
                def f():
                    if dh == 0:
                        slab_lists[idx] = []
                        slab_lists.pop(idx - 2, None)
                    slab = qslab.tile([128, 8, TC], PDT, tag="qslab", name="qslab")
                    nc.sync.dma_start(
                        out=slab,
                        in_=xq[
                            dh * 1024 : (dh + 1) * 1024,
                            idx * TC : (idx + 1) * TC,
                        ].rearrange("(n k) t -> k n t", k=128),
                    )
                    slab_lists[idx].append(slab)

                return f

            for tcx in range(NTC):
                t0 = tcx * TC
                with nc.named_scope("ropeq"):
                    qrope = pa.tile([128, NH, TC], F32R, tag="qrope", bufs=2)
                    for jh in range(NH):
                        rope(
                            qrope[:, jh, :], qraw_cur[:, jh, :], t0, TC, rotb_sb,
                            False,
                        )

                # ---- this chunk's filler units, in pop (≈execution) order.
                # qproj halves for chunk tcx+1 read slab_lists[tcx+1] at run
                # time; the prefetch unit preceding them in FIFO populates it.
                qunits = []
                if tcx + 1 < NTC:
                    qraw_next = qpool.tile([128, NH, TC], BF16, tag="qraw", name="qraw")
                    for jh in range(NH):
                        hold = [None]
                        qunits.append(
                            [
                                (lambda jh=jh, half=half, hold=hold: qproj_half(
                                    slab_lists[tcx + 1], hold, qraw_next, jh, half
                                ))
                                for half in range(2)
                            ]
                        )
                ounits = (
                    make_oproj_units(prev_attnT, prev_t0)
                    if prev_attnT is not None
                    else []
                )
                vunits = []
                if causal and 2 * (tcx + 1) < n_kch:
                    vunits = [
                        (lambda tch=tch: v_chunk(tch))
                        for tch in (2 * (tcx + 1), 2 * (tcx + 1) + 1)
                    ]

                xqu = (
                    [qslab_quarter_unit(tcx + 1, dh) for dh in range(4)]
                    if tcx + 1 < NTC
                    else []
                )
                if causal and tcx == 0:
                    kus = [
                        (lambda tch=tch: k_chunk(tch))
                        for tch in range(n_kpre, n_kch)
                    ]
                    order = [
                        (C_KCHUNK, kus[0]), (C_KCHUNK, kus[1]),
                        (0.0, xqu[0]), (0.0, xqu[1]),
                        (C_QPROJ_HALF, qunits[0][0]),
                        (C_KCHUNK, kus[2]),
                        (0.0, xqu[2]), (0.0, xqu[3]),
                        (C_QPROJ_HALF, qunits[0][1]),
                        (C_KCHUNK, kus[3]),
                        (C_QPROJ_HALF, qunits[1][0]),
                        (C_KCHUNK, kus[4]),
                        (C_QPROJ_HALF, qunits[1][1]),
                        (C_KCHUNK, kus[5]),
                        (C_QPROJ_HALF, qunits[2][0]),
                        (C_QPROJ_HALF, qunits[2][1]),
                        (C_VCHUNK, vunits[0]),
                        (C_QPROJ_HALF, qunits[3][0]),
                        (C_VCHUNK, vunits[1]),
                        (C_QPROJ_HALF, qunits[3][1]),
                    ]
                    for c, u in order:
                        pacer.add(c, u, tcx)
                else:
                    if tcx == 1:
                        for jh in range(NH):
                            pacer.add(
                                0.0,
                                lambda jh=jh: nc.sync.dma_start(
                                    out=wo_sb[:, jh, :],
                                    in_=wo[
                                        jh * 128 : (jh + 1) * 128, :
                                    ].rearrange("(n k) d -> k n d", k=128),
                                ),
                                tcx,
                            )
                    oi = 0
                    for r in range(4):
                        if r < len(qunits):
                            if r == 0 and xqu:
                                pacer.add(0.0, xqu[0], tcx)
                                pacer.add(0.0, xqu[1], tcx)
                            pacer.add(C_QPROJ_HALF, qunits[r][0], tcx)
                            if r == 0 and xqu:
                                pacer.add(0.0, xqu[2], tcx)
                                pacer.add(0.0, xqu[3], tcx)
                            pacer.add(C_QPROJ_HALF, qunits[r][1], tcx)
                        if r < len(vunits):
                            pacer.add(C_VCHUNK, vunits[r], tcx)
                        take = 8 if r < 3 else len(ounits) - oi
                        for u in ounits[oi : oi + take]:
                            pacer.add(C_OPROJ_UNIT, u, tcx + 1)
                        oi += take

                nt_valid = 4 * (tcx + 1) if causal else NTT
                n_off = nt_valid - 4 if causal else nt_valid
                ngroups_pe = nt_valid // GW * NH
                quantum = pacer.due_total(tcx) / max(ngroups_pe, 1)

                attnT = pa.tile([128, NH, TC], BF16, tag="attnT", bufs=3)
                for h in range(NH):
                    with nc.named_scope("attn"):
                        pv = pvps.tile([128, 4, 256], F32, tag="pv", name="pv_ps")
                        # off-diagonal groups (no masking at all)
                        for gg in range(n_off // GW):
                            qk = qkps.tile([128, GW, 512], F32, tag="qk", name="qk_ps")
                            for b in range(GW):
                                Tt = GW * gg + b
                                nc.tensor.matmul(
                                    qk[:, b, :],
                                    kT_rope[:, Tt * 128 : (Tt + 1) * 128],
                                    qrope[:, h, :],
                                    start=True,
                                    stop=True,
                                )
                            nc.scalar.activation(
                                out=qk, in_=qk, func=Tanh, scale=ATTN_MULT / CAP
                            )
                            pt = pa.tile([128, GW, TC], BF16, tag="pt", bufs=2, name="pt")
                            nc.scalar.activation(out=pt, in_=qk, func=Exp, scale=CAP)
                            pacer.tick(quantum)
                            for s4 in range(4):
                                for b in range(GW):
                                    Tt = GW * gg + b
                                    nc.tensor.matmul(
                                        pv[:, s4, 0:129],
                                        pt[:, b, s4 * 128 : (s4 + 1) * 128],
                                        vaug[:, Tt, 0:129],
                                        start=(gg == 0 and b == 0 and s4 % 2 == 0),
                                        stop=(
                                            not causal
                                            and gg == n_off // GW - 1
                                            and b == GW - 1
                                        ),
                                        skip_group_check=True,
                                    )
                        if causal:
                            # diagonal: 4 tiles, sliced to unmasked queries
                            for dg in range(2):
                                qk = qkps.tile(
                                    [128, GW, 512], F32, tag="qk", name="qk_ps"
                                )
                                pt = pa.tile(
                                    [128, GW, TC], BF16, tag="pt", bufs=2, name="pt"
                                )
                                for j in range(GW):
                                    b = GW * dg + j
                                    Tt = 4 * tcx + b
                                    q0 = 128 * b
                                    nc.tensor.matmul(
                                        qk[:, j, q0:512],
                                        kT_rope[:, Tt * 128 : (Tt + 1) * 128],
                                        qrope[:, h, q0:512],
                                        start=True,
                                        stop=True,
                                    )
                                for j in range(GW):
                                    b = GW * dg + j
                                    q0 = 128 * b
                                    nc.scalar.activation(
                                        out=qk[:, j, q0:512],
                                        in_=qk[:, j, q0:512],
                                        func=Tanh,
                                        scale=ATTN_MULT / CAP,
                                    )
                                    nc.scalar.activation(
                                        out=pt[:, j, q0:512],
                                        in_=qk[:, j, q0:512],
                                        func=Exp,
                                        scale=CAP,
                                    )
                                    nc.vector.tensor_mul(
                                        pt[:, j, q0 : q0 + 128],
                                        pt[:, j, q0 : q0 + 128],
                                        tri_sb,
                                    )
                                pacer.tick(quantum)
                                for s4 in range(GW * dg, 4):
                                    for j in range(GW):
                                        b = GW * dg + j
                                        if s4 < b:
                                            continue
                                        Tt = 4 * tcx + b
                                        nc.tensor.matmul(
                                            pv[:, s4, 0:129],
                                            pt[:, b % GW, s4 * 128 : (s4 + 1) * 128],
                                            vaug[:, Tt, 0:129],
                                            start=(
                                                tcx == 0
                                                and b == 0
                                                and s4 % 2 == 0
                                            ),
                                            stop=(b == s4),
                                            skip_group_check=True,
                                        )
                                with nc.named_scope("attn_fin"):
                                    for s4 in (GW * dg, GW * dg + 1):
                                        attn_fin(pv, attnT, h, s4)
                        else:
                            with nc.named_scope("attn_fin"):
                                for s4 in range(4):
                                    attn_fin(pv, attnT, h, s4)

                pacer.drain_due(tcx)
                if tcx + 1 < NTC:
                    qraw_cur = qraw_next
                prev_attnT, prev_t0 = attnT, t0

            pacer.drain_all()
            # tail: O proj of the last chunk
            for u in make_oproj_units(prev_attnT, prev_t0):
                u()

            pa.release()
            vrawp.release()
            ropepool.release()
            qpool.release()
            qslab.release()
            kvslab.release()
            wpool.release()

    nc.compile()
    return nc


def _host_constants(T: int):
    d = KEY_SIZE
    inv_freq = 1.0 / (10000.0 ** (np.arange(0, d, 2, dtype=np.float64) / d))  # [64]
    pos = np.arange(T, dtype=np.float64)
    phase_half = pos[None, :] * inv_freq[:, None]  # [64, T]
    phase = np.concatenate([phase_half, phase_half], axis=0)  # [128, T] (tiled)
    cosT = np.cos(phase).astype(np.float32)
    sinT = np.sin(phase).astype(np.float32)

    R = np.zeros((128, 128), dtype=np.float32)
    R[:64, 64:] = -np.eye(64, dtype=np.float32)
    R[64:, :64] = np.eye(64, dtype=np.float32)
    rot = np.ascontiguousarray(R.T)

    ident = np.eye(128, dtype=np.float32)

    # tri[p, qq] = 1 where key-in-tile p <= query-in-tile qq (keep), else 0
    tri = np.triu(np.ones((128, 128), dtype=np.float32)).astype(ml_dtypes.bfloat16)

    NTT = T // 128
    vbg = np.zeros((128, NTT, 4), dtype=ml_dtypes.bfloat16)
    vbg[:, :, 0] = 1.0
    return cosT, sinT, rot, ident, tri, vbg


_NC_CACHE: dict = {}
LAST_RESULT = None
_LAST_IN_MAPS = None


def kernel(query, key, value, mask, Wq, Wk, Wv, Wo):
    global LAST_RESULT, _LAST_IN_MAPS
    query = np.asarray(query)
    key = np.asarray(key)
    value = np.asarray(value)
    mask = np.asarray(mask)
    Wq = np.asarray(Wq, dtype=np.float32)
    Wk = np.asarray(Wk, dtype=np.float32)
    Wv = np.asarray(Wv, dtype=np.float32)
    Wo = np.asarray(Wo, dtype=np.float32)

    b, T, D = query.shape
    assert b == 1 and D == D_MODEL, (b, D)
    TC = 512

    m2 = np.asarray(mask).reshape(T, T).astype(bool)
    if np.array_equal(m2, np.tril(np.ones((T, T), dtype=bool))):
        causal = True
    elif m2.all():
        causal = False
    else:
        raise ValueError("unsupported mask pattern (expected causal or full)")

    kkey = (T, causal, PROJ_DT)
    if kkey not in _NC_CACHE:
        _NC_CACHE[kkey] = build_nc(T, causal)
    nc = _NC_CACHE[kkey]

    pnp = ml_dtypes.bfloat16 if PROJ_DT == "bf16" else np.float32
    xq = np.ascontiguousarray(query[0].T).astype(pnp)  # [D, T]
    xk = np.ascontiguousarray(key[0].T).astype(pnp)
    xv = np.ascontiguousarray(value[0].T).astype(pnp)
    cosT, sinT, rot, ident, tri, vbg = _host_constants(T)

    JW = NH * KEY_SIZE
    in_maps = []
    for c in range(N_CORES):
        in_maps.append(
            {
                "xq": xq,
                "xk": xk,
                "xv": xv,
                "wq": np.ascontiguousarray(
                    Wq[:, c * JW : (c + 1) * JW]
                    .reshape(D_MODEL // 128, 128, NH, 128)
                    .transpose(2, 1, 0, 3)
                ).astype(pnp),
                "wk": np.ascontiguousarray(
                    Wk[:, c * KEY_SIZE : (c + 1) * KEY_SIZE]
                    .reshape(D_MODEL // 128, 128, KEY_SIZE)
                    .transpose(1, 0, 2)
                ).astype(pnp),
                "wv": np.ascontiguousarray(
                    Wv[:, c * KEY_SIZE : (c + 1) * KEY_SIZE]
                    .reshape(D_MODEL // 128, 128, KEY_SIZE)
                    .transpose(1, 0, 2)
                ).astype(pnp),
                "wo": np.ascontiguousarray(Wo[c * JW : (c + 1) * JW, :]).astype(pnp),
                "cosT": cosT.astype(ml_dtypes.bfloat16),
                "sinT": sinT.astype(ml_dtypes.bfloat16),
                "rot": rot,
                "rotb": rot.astype(ml_dtypes.bfloat16),
                "identb": ident.astype(ml_dtypes.bfloat16),
                "tri": tri,
                "vbg": vbg,
            }
        )

    _LAST_IN_MAPS = in_maps
    trace = os.environ.get("MHA_TRACE") == "1"
    res = run_bass_kernel_spmd(nc, in_maps, list(range(N_CORES)), trace=trace)
    LAST_RESULT = res

    out = np.zeros((T, D), dtype=np.float64)
    for c in range(N_CORES):
        out += res.results[c]["out"].astype(np.float64)
    return out.astype(np.float32).reshape(1, T, D)


# revision 49
# speedup vs baseline: 1.0160x; 1.0160x over previous
"""Trainium2 Bass kernel for nn_MultiHeadAttention_83056077570808.

GQA multi-head attention (32 q heads, 8 kv heads, d_head=128, T=2048,
D=4096) with RoPE, tanh soft-capping at 30, causal mask, fp32 reference.

Sharding: tensor-parallel over heads across 8 cores. Core c owns kv head c
and q heads 4c..4c+3: Wq/Wk/Wv column-sharded, Wo row-sharded; activations
replicated. Each core computes a partial output (its heads' contribution
through its Wo rows); the host sums the 8 partials.

Schedule (v3; cost-model-guided, 515us baseline -> 440us):
  - the DMA engine is effectively a single serial resource (~330 B/ns for
    contiguous-run transfers), so every multi-MB load is split into ~1MB
    pieces emitted just-in-time in the same order the consuming PE units
    pop; weights are pre-rearranged on the HOST into the SBUF tile layout
    (head-major wq, k-major wk/wv) so their DMA runs are 8KB-contiguous
  - minimal prefix: K chunks 0..1 (+RoPE), V chunk 0, Q proj of chunk 0;
    K2..7 / V1..7 stream as pacer units inside the attention loop, the
    later K chunks deferred toward their causal deadlines to decongest
    the DMA-saturated first window
  - causal diagonal handled exactly: QK/tanh/exp sliced to the unmasked
    query range per 128-tile, a [128,128] triangular bf16 DVE multiply
    masks only the diagonal block (PV blocks fully under the diagonal are
    skipped), replacing the baseline's Pool-engine mask of whole groups
  - a deadline-tagged pacer interleaves filler units (Q proj half-chains,
    O proj (s4,nch) chains, K/V chunks, xq quarter-slab prefetches)
    BETWEEN each group's exp and PV so PE keeps running while ACT owns
    the tanh/exp chain; only units due this chunk count toward the tick
    budget, letting O proj drift into the ACT-bound late chunks
  - RoPE rotation matmul uses a bf16 rotation matrix for q (qraw held in
    bf16) and f32r for k; cos/sin tables are bf16

Per-core pipeline layouts (contraction dim = partition dim everywhere):
  - host supplies query/key/value pre-transposed as X^T [D, T] in bf16
  - K/V proj: kT/vT [dk, T] = Wk/Wv-tile.T @ X^T   (bf16 matmuls, fp32 acc)
  - RoPE on kT and qT via a 128x128 rotation matmul + cos/sin elementwise
  - V transposed on-PE to V [T, dk] bf16, augmented with a ones column so
    the PV matmul computes the softmax denominator for free
  - S^T[Tt, t] = kT_rope-tile.T @ qT_rope      (f32r)
    tanh in PSUM, exp to bf16 SBUF (ACT scales fused)
  - attn[t, 0:129] = P^T-slice.T @ [V | ones]  (bf16, accumulated over
    T tiles; col 128 = denominator); normalize, PE-transpose to attnT
  - O proj: out[t, :] += attnT-tile.T @ Wo-tile (bf16)

No max-subtraction in softmax: capping bounds logits to [-30, 30].
Masked logits never reach attn: fully-masked PV blocks are skipped and
the diagonal 128x128 block is zeroed in bf16 after exp.
"""

import collections
import os
import sys

for _p in ("/opt/trn_rl_repo", os.path.expanduser("~/.axon_site/_ro/trn_rl_repo")):
    if os.path.isdir(_p) and _p not in sys.path:
        sys.path.insert(0, _p)

import numpy as np
import ml_dtypes

import concourse.bass as bass
import concourse.tile as tile
from concourse import bacc, mybir
from concourse.bass_utils import run_bass_kernel_spmd

F32 = mybir.dt.float32
F32R = mybir.dt.float32r
BF16 = mybir.dt.bfloat16

D_MODEL = 4096
KEY_SIZE = 128
NUM_Q_HEADS = 32
NUM_KV_HEADS = 8
N_CORES = 8
NH = NUM_Q_HEADS // NUM_KV_HEADS  # q heads per core = 4
ATTN_MULT = 0.08838834764831845
CAP = 30.0

Tanh = mybir.ActivationFunctionType.Tanh
Exp = mybir.ActivationFunctionType.Exp

PROJ_DT = os.environ.get("MHA_PROJ_DT", "bf16")


class Pacer:
    """Deadline-tagged filler queue: units are (cost_ns, fn, deadline_tcx).

    tick(budget) pops FIFO until ~budget ns of unit cost has been emitted.
    drain_due(tcx) force-emits everything whose deadline has arrived.
    """

    def __init__(self):
        self.q = collections.deque()

    def add(self, cost, fn, deadline):
        self.q.append((cost, fn, deadline))

    def total(self):
        return sum(c for c, _, _ in self.q)

    def due_total(self, tcx):
        return sum(c for c, _, dl in self.q if dl <= tcx)

    def tick(self, budget):
        spent = 0.0
        while self.q and spent < budget:
            c, fn, _ = self.q.popleft()
            fn()
            spent += c
        return spent

    def drain_due(self, tcx):
        keep = collections.deque()
        while self.q:
            c, fn, dl = self.q.popleft()
            if dl <= tcx:
                fn()
            else:
                keep.append((c, fn, dl))
        self.q = keep

    def drain_all(self):
        while self.q:
            _, fn, _ = self.q.popleft()
            fn()


def build_nc(T: int, causal: bool, proj_dt: str = PROJ_DT):
    """Emit the Bass program for one core (SPMD: all cores run this)."""
    D = D_MODEL
    TC = 512                 # t-chunk width for attention
    NTC = T // TC            # t-chunks
    NTT = T // 128           # T tiles (key side)
    NDT = D // 128           # contraction tiles over d_model = 32
    JW = NH * KEY_SIZE       # per-core q/o width = 512
    GW = 2                   # T tiles per QK group (2 PSUM banks)
    PDT = BF16 if proj_dt == "bf16" else F32R

    # pacer unit cost estimates (ns, full-pstate PE)
    C_QPROJ_HALF = 3400.0
    C_OPROJ_UNIT = 900.0
    C_VCHUNK = 3000.0
    C_KCHUNK = 3600.0

    nc = bacc.Bacc(None, target_bir_lowering=False)

    xq = nc.dram_tensor("xq", [D, T], PDT, kind="ExternalInput")
    xk = nc.dram_tensor("xk", [D, T], PDT, kind="ExternalInput")
    xv = nc.dram_tensor("xv", [D, T], PDT, kind="ExternalInput")
    wq = nc.dram_tensor("wq", [NH, 128, D // 128, 128], PDT, kind="ExternalInput")
    wk = nc.dram_tensor("wk", [128, D // 128, KEY_SIZE], PDT, kind="ExternalInput")
    wv = nc.dram_tensor("wv", [128, D // 128, KEY_SIZE], PDT, kind="ExternalInput")
    wo = nc.dram_tensor("wo", [JW, D], PDT, kind="ExternalInput")
    cosd = nc.dram_tensor("cosT", [128, T], BF16, kind="ExternalInput")
    sind = nc.dram_tensor("sinT", [128, T], BF16, kind="ExternalInput")
    rotd = nc.dram_tensor("rot", [128, 128], F32R, kind="ExternalInput")
    rotbd = nc.dram_tensor("rotb", [128, 128], BF16, kind="ExternalInput")
    identbd = nc.dram_tensor("identb", [128, 128], BF16, kind="ExternalInput")
    trid = nc.dram_tensor("tri", [128, 128], BF16, kind="ExternalInput")
    vbgd = nc.dram_tensor("vbg", [128, NTT, 4], BF16, kind="ExternalInput")
    outd = nc.dram_tensor("out", [T, D], BF16, kind="ExternalOutput")

    with tile.TileContext(nc) as tc:
        with (
            tc.tile_pool(name="const", bufs=1) as constp,
            tc.tile_pool(name="persist", bufs=1) as persist,
            tc.tile_pool(name="qkps", bufs=2, space="PSUM") as qkps,
            tc.tile_pool(name="pvps", bufs=1, space="PSUM") as pvps,
            tc.tile_pool(name="mmps", bufs=2, space="PSUM") as mmps,
        ):
            rot_sb = constp.tile([128, 128], F32R)
            rotb_sb = constp.tile([128, 128], BF16)
            identb_sb = constp.tile([128, 128], BF16)
            tri_sb = constp.tile([128, 128], BF16)
            cos_sb = constp.tile([128, T], BF16)
            sin_sb = constp.tile([128, T], BF16)

            kT_rope = persist.tile([128, T], F32R)
            vaug = persist.tile([128, NTT, 132], BF16)

            wpool = tc.alloc_tile_pool(name="wpool", bufs=1)
            wq_sb = wpool.tile([128, NH, NDT, 128], PDT)
            wo_sb = wpool.tile([128, NH, D], PDT)
            wk_sb = wpool.tile([128, NDT, 128], PDT)
            wv_sb = wpool.tile([128, NDT, 128], PDT)

            kvslab = tc.alloc_tile_pool(
                name="kvslab", bufs=int(os.environ.get("MHA_KVSLABS", "5"))
            )
            qslab = tc.alloc_tile_pool(
                name="qslab", bufs=int(os.environ.get("MHA_QSLABS", "4"))
            )
            qpool = tc.alloc_tile_pool(name="qpool", bufs=2)
            ropepool = tc.alloc_tile_pool(
                name="ropetmp", bufs=int(os.environ.get("MHA_RTBUFS", "2"))
            )
            vrawp = tc.alloc_tile_pool(name="vraw", bufs=2)

            # ---------------- helpers
            def load_qslabs(t0):
                """4 quarter-slabs [128, 8, TC] covering all D rows of xq
                for t-chunk at t0."""
                slabs = []
                for dh in range(4):
                    slab = qslab.tile([128, 8, TC], PDT, tag="qslab", name="qslab")
                    nc.sync.dma_start(
                        out=slab,
                        in_=xq[
                            dh * 1024 : (dh + 1) * 1024, t0 : t0 + TC
                        ].rearrange("(n k) t -> k n t", k=128),
                    )
                    slabs.append(slab)
                return slabs

            def rope(dst, src, t0, tw, rtile, f32r_src):
                """dst[128, tw] (f32r) = RoPE(src[128, tw]) at positions t0..

                In-place safe (dst may alias src): src is fully read by the
                rotation matmul and the cos-mul before dst is written."""
                rp = mmps.tile([128, 512], F32, tag="mm", name="rope_ps")
                nc.tensor.matmul(rp[:, :tw], rtile, src, start=True, stop=True)
                t1 = ropepool.tile([128, 512], F32, tag="rt", name="rope_t1")
                nc.gpsimd.tensor_mul(
                    t1[:, :tw],
                    src.bitcast(F32) if f32r_src else src,
                    cos_sb[:, t0 : t0 + tw],
                )
                t2 = ropepool.tile([128, 512], F32, tag="rt", name="rope_t2")
                nc.vector.tensor_mul(t2[:, :tw], rp[:, :tw], sin_sb[:, t0 : t0 + tw])
                nc.vector.tensor_add(dst, t1[:, :tw], t2[:, :tw])

            def kv_chunk(w_sb, xsrc, tch, dest_slice):
                """One 256-col projection chunk of K or V into dest slice."""
                ps = mmps.tile([128, 512], F32, tag="mm", name="kv_ps")
                for dh in range(4):
                    slab = kvslab.tile([128, 8, 256], PDT, tag="slab", name="kvslab")
                    nc.sync.dma_start(
                        out=slab,
                        in_=xsrc[
                            dh * 1024 : (dh + 1) * 1024,
                            tch * 256 : (tch + 1) * 256,
                        ].rearrange("(n k) t -> k n t", k=128),
                    )
                    for i in range(8):
                        nc.tensor.matmul(
                            ps[:, :256],
                            w_sb[:, dh * 8 + i, :],
                            slab[:, i, :],
                            start=(dh == 0 and i == 0),
                            stop=(dh == 3 and i == 7),
                        )
                nc.scalar.copy(out=dest_slice, in_=ps[:, :256])

            def k_chunk(tch):
                """K proj chunk into kT_rope (raw); rope runs in place after
                each odd chunk (pair 0's rope is emitted explicitly later so
                the first QK projections can slot in before cos/sin land)."""
                with nc.named_scope("kproj"):
                    kv_chunk(wk_sb, xk, tch, kT_rope[:, tch * 256 : (tch + 1) * 256])
                    if tch % 2 == 1 and tch >= 3:
                        ch = tch // 2
                        sl = kT_rope[:, ch * TC : (ch + 1) * TC]
                        rope(sl, sl, ch * TC, TC, rot_sb, True)

            def v_chunk(tch):
                """V proj chunk tch (256 cols) -> transpose -> vaug tiles."""
                with nc.named_scope("vproj"):
                    vraw = vrawp.tile([128, 256], BF16, tag="vr", name="vraw")
                    kv_chunk(wv_sb, xv, tch, vraw)
                    for b in (2 * tch, 2 * tch + 1):
                        tp = mmps.tile([128, 512], BF16, tag="mm", name="vtr_ps")
                        nc.tensor.transpose(
                            tp[:, :128],
                            vraw[:, (b % 2) * 128 : (b % 2 + 1) * 128],
                            identb_sb,
                        )
                        nc.vector.tensor_copy(vaug[:, b, 0:128], tp[:, :128])

            def qproj_half(slabs, ps_holder, qraw, jh, half):
                """Half a Q-proj chain: 16 matmuls (2 quarter-slabs)."""
                with nc.named_scope("qproj"):
                    if half == 0:
                        ps_holder[0] = mmps.tile([128, 512], F32, tag="mm", name="q_ps")
                    ps = ps_holder[0]
                    for dq in (2 * half, 2 * half + 1):
                        for i in range(8):
                            nc.tensor.matmul(
                                ps,
                                wq_sb[:, jh, dq * 8 + i, :],
                                slabs[dq][:, i, :],
                                start=(dq == 0 and i == 0),
                                stop=(dq == 3 and i == 7),
                            )
                    if half == 1:
                        nc.vector.tensor_copy(qraw[:, jh, :], ps)

            def qproj_full(slabs, qraw, jh, psum_pool=None):
                with nc.named_scope("qproj"):
                    if psum_pool is None:
                        ps = mmps.tile([128, 512], F32, tag="mm", name="q_ps")
                    else:
                        ps = psum_pool.tile([128, 2, 512], F32, tag="qk", name="q_ps")[
                            :, 0, :
                        ]
                    for dq in range(4):
                        for i in range(8):
                            nc.tensor.matmul(
                                ps,
                                wq_sb[:, jh, dq * 8 + i, :],
                                slabs[dq][:, i, :],
                                start=(dq == 0 and i == 0),
                                stop=(dq == 3 and i == 7),
                            )
                    nc.vector.tensor_copy(qraw[:, jh, :], ps)

            # ---------------- phase A prefix: K chunks 0..3 (+RoPE-k 0,1),
            # V chunks 0,1, Q proj(tc0). DMA emission order front-loads what
            # unlocks the most PE work per byte.
            for dh in range(4):
                nc.sync.dma_start(
                    out=wk_sb[:, dh * 8 : (dh + 1) * 8, :],
                    in_=wk[:, dh * 8 : (dh + 1) * 8, :],
                )
            qraw0 = qpool.tile([128, NH, TC], BF16, tag="qraw", name="qraw")

            n_kch = T // 256
            n_kpre = 2 if causal else n_kch
            n_vpre = 1 if causal else n_kch

            def load_wq_head(jh):
                nc.sync.dma_start(out=wq_sb[:, jh], in_=wq[jh])

            k_chunk(0)
            k_chunk(1)
            load_wq_head(0)
            slabs0 = load_qslabs(0)
            load_wq_head(1)
            nc.sync.dma_start(out=rot_sb, in_=rotd[:])
            nc.sync.dma_start(out=rotb_sb, in_=rotbd[:])
            nc.sync.dma_start(out=cos_sb, in_=cosd[:])
            nc.sync.dma_start(out=sin_sb, in_=sind[:])
            nc.sync.dma_start(out=identb_sb, in_=identbd[:])
            nc.sync.dma_start(out=tri_sb, in_=trid[:])
            nc.sync.dma_start(out=vaug[:, :, 128:132], in_=vbgd[:])
            for tch in range(2, n_kpre):
                k_chunk(tch)
            qproj_full(slabs0, qraw0, 0, psum_pool=qkps)
            load_wq_head(2)
            qproj_full(slabs0, qraw0, 1, psum_pool=qkps)
            load_wq_head(3)
            rope(kT_rope[:, 0:TC], kT_rope[:, 0:TC], 0, TC, rot_sb, True)
            nc.sync.dma_start(out=wv_sb, in_=wv[:])
            qproj_full(slabs0, qraw0, 2, psum_pool=qkps)
            v_chunk(0)
            qproj_full(slabs0, qraw0, 3, psum_pool=qkps)
            for tch in range(1, n_vpre):
                v_chunk(tch)

            # ---------------- main loop
            nc.sync.dma_start(out=wo_sb, in_=wo.rearrange("(n k) d -> k n d", k=128))
            pacer = Pacer()
            pa = tc.alloc_tile_pool(name="pa", bufs=1)

            def make_oproj_units(attnT, t0):
                """O proj of a chunk: 32 (s4,nch) units, 4 head-matmuls each;
                osb half-rows [128, 2048] DMAed per 4 nch."""
                units = []
                osb_rows = [None] * 4

                def unit(s4, nch):
                    def f():
                        with nc.named_scope("oproj"):
                            if nch % 4 == 0:
                                osb_rows[s4] = pa.tile(
                                    [128, D // 2], BF16, tag="osb", bufs=2, name="osb"
                                )
                            ps = mmps.tile([128, 512], F32, tag="mm", name="o_ps")
                            for jh in range(NH):
                                nc.tensor.matmul(
                                    ps,
                                    attnT[:, jh, s4 * 128 : (s4 + 1) * 128],
                                    wo_sb[:, jh, nch * 512 : (nch + 1) * 512],
                                    start=(jh == 0),
                                    stop=(jh == NH - 1),
                                )
                            nc.vector.tensor_copy(
                                osb_rows[s4][:, (nch % 4) * 512 : (nch % 4 + 1) * 512],
                                ps,
                            )
                            if nch % 4 == 3:
                                half = nch // 4
                                nc.sync.dma_start(
                                    out=outd[
                                        t0 + s4 * 128 : t0 + (s4 + 1) * 128,
                                        half * (D // 2) : (half + 1) * (D // 2),
                                    ],
                                    in_=osb_rows[s4],
                                )

                    return f

                for s4 in range(4):
                    for nch in range(D // 512):
                        units.append(unit(s4, nch))
                return units

            def attn_fin(pv, attnT, h, s4):
                rc = pa.tile([128, 1], F32, tag="rc", bufs=4, name="rc")
                nc.vector.reciprocal(rc, pv[:, s4, 128:129])
                an = pa.tile([128, 128], BF16, tag="an", bufs=2, name="an")
                nc.vector.tensor_scalar_mul(an, pv[:, s4, 0:128], rc)
                tp = mmps.tile([128, 512], BF16, tag="mm", name="atr")
                nc.tensor.transpose(tp[:, :128], an, identb_sb)
                nc.vector.tensor_copy(
                    attnT[:, h, s4 * 128 : (s4 + 1) * 128], tp[:, :128]
                )

            qraw_cur = qraw0
            prev_attnT = None
            prev_t0 = 0
            slab_lists = {0: slabs0}

            def qslab_quarter_unit(idx, dh):
                def f():
                    if dh == 0:
                        slab_lists[idx] = []
                        slab_lists.pop(idx - 2, None)
                    slab = qslab.tile([128, 8, TC], PDT, tag="qslab", name="qslab")
                    nc.sync.dma_start(
                        out=slab,
                        in_=xq[
                            dh * 1024 : (dh + 1) * 1024,
                            idx * TC : (idx + 1) * TC,
                        ].rearrange("(n k) t -> k n t", k=128),
                    )
                    slab_lists[idx].append(slab)

                return f

            for tcx in range(NTC):
                t0 = tcx * TC
                with nc.named_scope("ropeq"):
                    qrope = pa.tile([128, NH, TC], F32R, tag="qrope", bufs=2)
                    for jh in range(NH):
                        rope(
                            qrope[:, jh, :], qraw_cur[:, jh, :], t0, TC, rotb_sb,
                            False,
                        )

                # ---- this chunk's filler units, in pop (≈execution) order.
                # qproj halves for chunk tcx+1 read slab_lists[tcx+1] at run
                # time; the prefetch unit preceding them in FIFO populates it.
                qunits = []
                if tcx + 1 < NTC:
                    qraw_next = qpool.tile([128, NH, TC], BF16, tag="qraw", name="qraw")
                    for jh in range(NH):
                        hold = [None]
                        qunits.append(
                            [
                                (lambda jh=jh, half=half, hold=hold: qproj_half(
                                    slab_lists[tcx + 1], hold, qraw_next, jh, half
                                ))
                                for half in range(2)
                            ]
                        )
                ounits = (
                    make_oproj_units(prev_attnT, prev_t0)
                    if prev_attnT is not None
                    else []
                )
                vunits = []
                if causal and 2 * (tcx + 1) < n_kch:
                    vunits = [
                        (lambda tch=tch: v_chunk(tch))
                        for tch in (2 * (tcx + 1), 2 * (tcx + 1) + 1)
                    ]
                if causal and tcx == 0:
                    vunits.insert(0, lambda: v_chunk(1))

                xqu = (
                    [qslab_quarter_unit(tcx + 1, dh) for dh in range(4)]
                    if tcx + 1 < NTC
                    else []
                )
                if causal and tcx == 0:
                    kus = [
                        (lambda tch=tch: k_chunk(tch))
                        for tch in range(n_kpre, n_kch)
                    ]
                    order = [
                        (C_VCHUNK, vunits[0]),
                        (C_KCHUNK, kus[0]), (C_KCHUNK, kus[1]),
                        (0.0, xqu[0]), (0.0, xqu[1]),
                        (C_QPROJ_HALF, qunits[0][0]),
                        (0.0, xqu[2]), (0.0, xqu[3]),
                        (C_QPROJ_HALF, qunits[0][1]),
                        (C_QPROJ_HALF, qunits[1][0]),
                        (C_QPROJ_HALF, qunits[1][1]),
                        (C_QPROJ_HALF, qunits[2][0]),
                        (C_QPROJ_HALF, qunits[2][1]),
                        (C_VCHUNK, vunits[1]),
                        (C_QPROJ_HALF, qunits[3][0]),
                        (C_VCHUNK, vunits[2]),
                        (C_QPROJ_HALF, qunits[3][1]),
                    ]
                    for c, u in order:
                        pacer.add(c, u, tcx)
                    pacer.add(C_KCHUNK, kus[2], 1)
                    pacer.add(C_KCHUNK, kus[3], 1)
                    pacer.add(C_KCHUNK, kus[4], 2)
                    pacer.add(C_KCHUNK, kus[5], 2)
                else:
                    if tcx == 1:
                        for jh in range(NH):
                            pacer.add(
                                0.0,
                                lambda jh=jh: nc.sync.dma_start(
                                    out=wo_sb[:, jh, :],
                                    in_=wo[
                                        jh * 128 : (jh + 1) * 128, :
                                    ].rearrange("(n k) d -> k n d", k=128),
                                ),
                                tcx,
                            )
                    oi = 0
                    for r in range(4):
                        if r < len(qunits):
                            if r == 0 and xqu:
                                pacer.add(0.0, xqu[0], tcx)
                                pacer.add(0.0, xqu[1], tcx)
                            pacer.add(C_QPROJ_HALF, qunits[r][0], tcx)
                            if r == 0 and xqu:
                                pacer.add(0.0, xqu[2], tcx)
                                pacer.add(0.0, xqu[3], tcx)
                            pacer.add(C_QPROJ_HALF, qunits[r][1], tcx)
                        if r < len(vunits):
                            pacer.add(C_VCHUNK, vunits[r], tcx)
                        take = 8 if r < 3 else len(ounits) - oi
                        for u in ounits[oi : oi + take]:
                            pacer.add(C_OPROJ_UNIT, u, tcx + 1)
                        oi += take

                nt_valid = 4 * (tcx + 1) if causal else NTT
                n_off = nt_valid - 4 if causal else nt_valid
                ngroups_pe = nt_valid // GW * NH
                quantum = pacer.due_total(tcx) / max(ngroups_pe, 1)

                attnT = pa.tile([128, NH, TC], BF16, tag="attnT", bufs=3)
                for h in range(NH):
                    with nc.named_scope("attn"):
                        pv = pvps.tile([128, 4, 256], F32, tag="pv", name="pv_ps")
                        # off-diagonal groups (no masking at all)
                        for gg in range(n_off // GW):
                            qk = qkps.tile([128, GW, 512], F32, tag="qk", name="qk_ps")
                            for b in range(GW):
                                Tt = GW * gg + b
                                nc.tensor.matmul(
                                    qk[:, b, :],
                                    kT_rope[:, Tt * 128 : (Tt + 1) * 128],
                                    qrope[:, h, :],
                                    start=True,
                                    stop=True,
                                )
                            nc.scalar.activation(
                                out=qk, in_=qk, func=Tanh, scale=ATTN_MULT / CAP
                            )
                            pt = pa.tile([128, GW, TC], BF16, tag="pt", bufs=2, name="pt")
                            nc.scalar.activation(out=pt, in_=qk, func=Exp, scale=CAP)
                            pacer.tick(quantum)
                            for s4 in range(4):
                                for b in range(GW):
                                    Tt = GW * gg + b
                                    nc.tensor.matmul(
                                        pv[:, s4, 0:129],
                                        pt[:, b, s4 * 128 : (s4 + 1) * 128],
                                        vaug[:, Tt, 0:129],
                                        start=(gg == 0 and b == 0 and s4 % 2 == 0),
                                        stop=(
                                            not causal
                                            and gg == n_off // GW - 1
                                            and b == GW - 1
                                        ),
                                        skip_group_check=True,
                                    )
                        if causal:
                            # diagonal: 4 tiles, sliced to unmasked queries
                            for dg in range(2):
                                qk = qkps.tile(
                                    [128, GW, 512], F32, tag="qk", name="qk_ps"
                                )
                                pt = pa.tile(
                                    [128, GW, TC], BF16, tag="pt", bufs=2, name="pt"
                                )
                                for j in range(GW):
                                    b = GW * dg + j
                                    Tt = 4 * tcx + b
                                    q0 = 128 * b
                                    nc.tensor.matmul(
                                        qk[:, j, q0:512],
                                        kT_rope[:, Tt * 128 : (Tt + 1) * 128],
                                        qrope[:, h, q0:512],
                                        start=True,
                                        stop=True,
                                    )
                                for j in range(GW):
                                    b = GW * dg + j
                                    q0 = 128 * b
                                    nc.scalar.activation(
                                        out=qk[:, j, q0:512],
                                        in_=qk[:, j, q0:512],
                                        func=Tanh,
                                        scale=ATTN_MULT / CAP,
                                    )
                                    nc.scalar.activation(
                                        out=pt[:, j, q0:512],
                                        in_=qk[:, j, q0:512],
                                        func=Exp,
                                        scale=CAP,
                                    )
                                    nc.vector.tensor_mul(
                                        pt[:, j, q0 : q0 + 128],
                                        pt[:, j, q0 : q0 + 128],
                                        tri_sb,
                                    )
                                pacer.tick(quantum)
                                for s4 in range(GW * dg, 4):
                                    for j in range(GW):
                                        b = GW * dg + j
                                        if s4 < b:
                                            continue
                                        Tt = 4 * tcx + b
                                        nc.tensor.matmul(
                                            pv[:, s4, 0:129],
                                            pt[:, b % GW, s4 * 128 : (s4 + 1) * 128],
                                            vaug[:, Tt, 0:129],
                                            start=(
                                                tcx == 0
                                                and b == 0
                                                and s4 % 2 == 0
                                            ),
                                            stop=(b == s4),
                                            skip_group_check=True,
                                        )
                                with nc.named_scope("attn_fin"):
                                    for s4 in (GW * dg, GW * dg + 1):
                                        attn_fin(pv, attnT, h, s4)
                        else:
                            with nc.named_scope("attn_fin"):
                                for s4 in range(4):
                                    attn_fin(pv, attnT, h, s4)

                pacer.drain_due(tcx)
                if tcx + 1 < NTC:
                    qraw_cur = qraw_next
                prev_attnT, prev_t0 = attnT, t0

            pacer.drain_all()
            # tail: O proj of the last chunk
            for u in make_oproj_units(prev_attnT, prev_t0):
                u()

            pa.release()
            vrawp.release()
            ropepool.release()
            qpool.release()
            qslab.release()
            kvslab.release()
            wpool.release()

    nc.compile()
    return nc


def _host_constants(T: int):
    d = KEY_SIZE
    inv_freq = 1.0 / (10000.0 ** (np.arange(0, d, 2, dtype=np.float64) / d))  # [64]
    pos = np.arange(T, dtype=np.float64)
    phase_half = pos[None, :] * inv_freq[:, None]  # [64, T]
    phase = np.concatenate([phase_half, phase_half], axis=0)  # [128, T] (tiled)
    cosT = np.cos(phase).astype(np.float32)
    sinT = np.sin(phase).astype(np.float32)

    R = np.zeros((128, 128), dtype=np.float32)
    R[:64, 64:] = -np.eye(64, dtype=np.float32)
    R[64:, :64] = np.eye(64, dtype=np.float32)
    rot = np.ascontiguousarray(R.T)

    ident = np.eye(128, dtype=np.float32)

    # tri[p, qq] = 1 where key-in-tile p <= query-in-tile qq (keep), else 0
    tri = np.triu(np.ones((128, 128), dtype=np.float32)).astype(ml_dtypes.bfloat16)

    NTT = T // 128
    vbg = np.zeros((128, NTT, 4), dtype=ml_dtypes.bfloat16)
    vbg[:, :, 0] = 1.0
    return cosT, sinT, rot, ident, tri, vbg


_NC_CACHE: dict = {}
LAST_RESULT = None
_LAST_IN_MAPS = None


def kernel(query, key, value, mask, Wq, Wk, Wv, Wo):
    global LAST_RESULT, _LAST_IN_MAPS
    query = np.asarray(query)
    key = np.asarray(key)
    value = np.asarray(value)
    mask = np.asarray(mask)
    Wq = np.asarray(Wq, dtype=np.float32)
    Wk = np.asarray(Wk, dtype=np.float32)
    Wv = np.asarray(Wv, dtype=np.float32)
    Wo = np.asarray(Wo, dtype=np.float32)

    b, T, D = query.shape
    assert b == 1 and D == D_MODEL, (b, D)
    TC = 512

    m2 = np.asarray(mask).reshape(T, T).astype(bool)
    if np.array_equal(m2, np.tril(np.ones((T, T), dtype=bool))):
        causal = True
    elif m2.all():
        causal = False
    else:
        raise ValueError("unsupported mask pattern (expected causal or full)")

    kkey = (T, causal, PROJ_DT)
    if kkey not in _NC_CACHE:
        _NC_CACHE[kkey] = build_nc(T, causal)
    nc = _NC_CACHE[kkey]

    pnp = ml_dtypes.bfloat16 if PROJ_DT == "bf16" else np.float32
    xq = np.ascontiguousarray(query[0].T).astype(pnp)  # [D, T]
    xk = np.ascontiguousarray(key[0].T).astype(pnp)
    xv = np.ascontiguousarray(value[0].T).astype(pnp)
    cosT, sinT, rot, ident, tri, vbg = _host_constants(T)

    JW = NH * KEY_SIZE
    in_maps = []
    for c in range(N_CORES):
        in_maps.append(
            {
                "xq": xq,
                "xk": xk,
                "xv": xv,
                "wq": np.ascontiguousarray(
                    Wq[:, c * JW : (c + 1) * JW]
                    .reshape(D_MODEL // 128, 128, NH, 128)
                    .transpose(2, 1, 0, 3)
                ).astype(pnp),
                "wk": np.ascontiguousarray(
                    Wk[:, c * KEY_SIZE : (c + 1) * KEY_SIZE]
                    .reshape(D_MODEL // 128, 128, KEY_SIZE)
                    .transpose(1, 0, 2)
                ).astype(pnp),
                "wv": np.ascontiguousarray(
                    Wv[:, c * KEY_SIZE : (c + 1) * KEY_SIZE]
                    .reshape(D_MODEL // 128, 128, KEY_SIZE)
                    .transpose(1, 0, 2)
                ).astype(pnp),
                "wo": np.ascontiguousarray(Wo[c * JW : (c + 1) * JW, :]).astype(pnp),
                "cosT": cosT.astype(ml_dtypes.bfloat16),
                "sinT": sinT.astype(ml_dtypes.bfloat16),
                "rot": rot,
                "rotb": rot.astype(ml_dtypes.bfloat16),
                "identb": ident.astype(ml_dtypes.bfloat16),
                "tri": tri,
                "vbg": vbg,
            }
        )

    _LAST_IN_MAPS = in_maps
    trace = os.environ.get("MHA_TRACE") == "1"
    res = run_bass_kernel_spmd(nc, in_maps, list(range(N_CORES)), trace=trace)
    LAST_RESULT = res

    out = np.zeros((T, D), dtype=np.float64)
    for c in range(N_CORES):
        out += res.results[c]["out"].astype(np.float64)
    return out.astype(np.float32).reshape(1, T, D)
